# revision 9
# baseline (speedup 1.0000x reference)
"""Trainium2 Bass kernel for nn_BasicBlock (binary activation + binarized
weight-standardized 3x3 conv + residual + PReLU).

Contract: kernel(**inputs) takes FULL unsharded numpy inputs (keys as in
setup_inputs) and returns the FULL [32, 512, 28, 28] float32 output.
Internally shards the batch dim across 8 NeuronCores (4 images each); the
small conv weight + per-channel vectors are replicated.

Key math facts exploited:
- forward activations are sign(x*beta+b0) in {-1,0,1} and forward weights
  are sf[o]*gain[o]*sign(w_std) with sign in {-1,0,1}, so the conv
  contraction is exact in fp8/bf16 (products are +-1, fp32 PSUM
  accumulation of integers <= 4608); the per-channel scalar alpha*sf*gain
  folds into the epilogue.
- fp8e4 DoubleRow packs two contraction rows per PE cell (2 cin chunks per
  matmul), halving the matmul count. Conv matmul spacing (~178ns for
  N=420) is at the DoubleRow streaming roofline.

Schedule notes (from perfetto trace of the previous revision):
- every dma_start costs ~600ns of sequencer issue time -> batch DMAs and
  keep them off engines with early critical work (ACT does the signs).
- DMA fabric is ~358 GB/s aggregate; w-chunk0 (2.25MB) gates the first
  matmul, so it goes first, then x images in consumption order.
- act-pad memsets are border-only (interior is overwritten by xsign).
- image-0 activation signs run on DVE (is_ge + affine) so they do not
  queue behind the m0 weight signs on ACT.
- lhsT transposes+casts for chunk m+1 are emitted mid/end of conv m; the
  9 PSUM->SBUF casts alternate vector/scalar so they keep up with the PE
  transpose burst.
- per-(m,n) output staging: one 784-col DMA instead of two 14x28 ones.
"""

import numpy as np

import concourse.bass as bass
import concourse.mybir as mybir
import concourse.tile as tile
from concourse import bacc
from concourse.masks import make_identity

# problem constants (hardcoded per harness contract)
N_CORES = 8
N_PER = 4          # images per core (32 / 8)
C = 512            # Cin == Cout
H = W = 28
HP = WP = 30       # zero-padded spatial
TAPS = 9
KFAN = C * TAPS    # 4608 = fan-in per output channel
ALPHA = 0.2
BETA = 1.0
EPS = 1e-5
WS_SCALE = 1.0 / float(np.sqrt(KFAN))  # fan_in**-0.5
NCH = C // 128     # 4 channel chunks of 128
NPAIR = NCH // 2   # 2 DoubleRow pairs of chunks
ROWS_PER_TILE = 14 # output rows per matmul tile
NSPAT = H // ROWS_PER_TILE  # 2 spatial tiles per image
NFREE = ROWS_PER_TILE * WP  # 420: contiguous run incl. 2 pad cols per row
ACT_IMG = 912  # padded 30x30 image (900) + 12 slack: %16==0 for DoubleRow,
               # and covers the last tile's 420-run overhang (482+420=902)

FP32 = mybir.dt.float32
BF16 = mybir.dt.bfloat16
FP8 = mybir.dt.float8e4


def build_program():
    nc = bacc.Bacc(
        "TRN2",
        target_bir_lowering=False,
        debug=False,
        num_devices=1,
        num_swdge_queues=4,
    )
    x_h = nc.declare_dram_parameter("x", [N_PER, C, H, W], FP32, isOutput=False)
    w_h = nc.declare_dram_parameter("conv_weight", [C, C, 3, 3], FP32, isOutput=False)
    gain_h = nc.declare_dram_parameter("gain", [C], FP32, isOutput=False)
    b0_h = nc.declare_dram_parameter("move0_bias", [C], FP32, isOutput=False)
    b1_h = nc.declare_dram_parameter("move1_bias", [C], FP32, isOutput=False)
    pa_h = nc.declare_dram_parameter("prelu_a", [C], FP32, isOutput=False)
    b2_h = nc.declare_dram_parameter("move2_bias", [C], FP32, isOutput=False)
    out_h = nc.declare_dram_parameter("out", [N_PER, C, H, W], FP32, isOutput=True)

    x_ap = x_h[:, :, :, :]
    w_ap = w_h[:, :, :, :]
    out_ap = out_h[:, :, :, :]

    with tile.TileContext(nc) as tc:
        with (
            tc.tile_pool(name="persist", bufs=1) as persist,
            tc.tile_pool(name="scratch", bufs=2) as scratch,
            tc.tile_pool(name="stats", bufs=4) as stats,
            tc.tile_pool(name="epi", bufs=3) as epi,
            tc.tile_pool(name="stage", bufs=3) as stage,
            tc.tile_pool(name="psum_mm", bufs=4, space="PSUM") as psum_mm,
            tc.tile_pool(name="psum_tr", bufs=3, space="PSUM") as psum_tr,
        ):
            # ---- activation storage: one tile per DoubleRow pair ---------
            # act_q[q] : [128, 2(half), N_PER, ACT_IMG] fp8, zero border.
            act_q = [
                persist.tile(
                    [128, 2, N_PER, ACT_IMG], FP8, tag=f"act{q}", name=f"act{q}"
                )
                for q in range(NPAIR)
            ]

            # border-only memsets (interior is overwritten by xsign).
            # image layout is a 30x30 row-major grid at [..., n, 0:900]:
            #   [0:31]    top pad row + (1,0) left pad
            #   (r,29),(r+1,0) adjacent pairs for r=1..28 -> strided 2-runs
            #   [869:912] (28,29) right pad + bottom pad row + slack
            def act_border_zero(n):
                for q in range(NPAIR):
                    t = act_q[q]
                    nc.gpsimd.memset(t[:, :, n, 0:31], 0.0)
                    mid = t[:, :, n, 59:899].rearrange(
                        "p a (r w) -> p a r w", w=WP
                    )[:, :, :, 0:2]
                    nc.gpsimd.memset(mid, 0.0)
                    nc.gpsimd.memset(t[:, :, n, 869:912], 0.0)

            for n in range(N_PER):
                act_border_zero(n)

            # ---- small per-channel vectors: one DMA each, [128, NCH] -----
            def load_vec(h, nm):
                t = persist.tile([128, NCH], FP32, tag=nm, name=nm)
                nc.gpsimd.dma_start(
                    out=t, in_=h[:].rearrange("(c p) -> p c", p=128)
                )
                return t

            gain_t = load_vec(gain_h, "gain")
            b0_t = load_vec(b0_h, "b0")
            b1_t = load_vec(b1_h, "b1")
            pa_t = load_vec(pa_h, "pa")
            b2_t = load_vec(b2_h, "b2")
            gain_c = [gain_t[:, c : c + 1] for c in range(NCH)]
            b0_c = [b0_t[:, c : c + 1] for c in range(NCH)]
            b1_c = [b1_t[:, c : c + 1] for c in range(NCH)]
            pa_c = [pa_t[:, c : c + 1] for c in range(NCH)]

            ident = persist.tile([128, 128], BF16, tag="ident")
            make_identity(nc, ident)

            # derived per-channel epilogue constants (batched over chunks):
            #   one_minus_a = 1 - prelu_a ; ab1b2 = prelu_a*move1_bias + b2
            oma_t = persist.tile([128, NCH], FP32, tag="oma", name="oma")
            nc.vector.tensor_scalar(
                out=oma_t, in0=pa_t, scalar1=-1.0, scalar2=1.0,
                op0=mybir.AluOpType.mult, op1=mybir.AluOpType.add,
            )
            ab_t = persist.tile([128, NCH], FP32, tag="ab1b2", name="ab1b2")
            nc.vector.tensor_tensor(
                out=ab_t, in0=pa_t, in1=b1_t, op=mybir.AluOpType.mult
            )
            nc.vector.tensor_tensor(
                out=ab_t, in0=ab_t, in1=b2_t, op=mybir.AluOpType.add
            )
            negb0_t = persist.tile([128, NCH], FP32, tag="negb0", name="negb0")
            nc.vector.tensor_scalar_mul(out=negb0_t, in0=b0_t, scalar1=-1.0)
            one_minus_a = [oma_t[:, c : c + 1] for c in range(NCH)]
            ab1b2 = [ab_t[:, c : c + 1] for c in range(NCH)]
            negb0_c = [negb0_t[:, c : c + 1] for c in range(NCH)]

            # ---- input DMAs, all on the sync queue in fabric-consumption
            # order: w0 (3 pieces, stats pipeline per piece), image 0..3,
            # then w1..w3.  Every dma_start costs ~600ns of issue time, so
            # images 1..3 still go as [128,784] singles (consumption order
            # beats batched c-chunk transfers here).
            w_flat = w_ap.rearrange("o i a b -> o (i a b)")
            w_tiles = {}

            def w_dma(m, pieces):
                wt = scratch.tile([128, KFAN], FP32, tag="wtile", name=f"wt{m}")
                step = KFAN // pieces
                for j in range(pieces):
                    nc.sync.dma_start(
                        out=wt[:, j * step : (j + 1) * step],
                        in_=w_flat[
                            m * 128 : (m + 1) * 128, j * step : (j + 1) * step
                        ],
                    )
                w_tiles[m] = wt

            xs_tiles = [
                persist.tile(
                    [128, N_PER, H, W], FP32, tag=f"xs{c}", name=f"xs{c}"
                )
                for c in range(NCH)
            ]
            xr = x_ap.rearrange("n c h w -> c n h w")

            def x_dma(n):
                for c in range(NCH):
                    nc.sync.dma_start(
                        out=xs_tiles[c][:, n],
                        in_=xr[c * 128 : (c + 1) * 128, n],
                    )

            w_dma(0, 3)
            for n in range(N_PER):
                x_dma(n)
            for m in range(1, NCH):
                w_dma(m, 1)

            # ---- activation signs ----------------------------------------
            # image 0 chunks run on DVE (is_ge + affine) so ACT is free for
            # the m0 weight signs; everything else on ACT.
            def xsign_act(n, c):
                dst = act_q[c // 2][:, c % 2, n, : HP * WP].rearrange(
                    "p (h w) -> p h w", w=WP
                )[:, 1 : 1 + H, 1 : 1 + W]
                nc.scalar.activation(
                    out=dst,
                    in_=xs_tiles[c][:, n],
                    func=mybir.ActivationFunctionType.Sign,
                    bias=b0_c[c],
                    scale=BETA,
                )

            def xsign_dve(n, c):
                u = stats.tile([128, H * W], FP32, tag="xsu", name="xsu")
                xin = xs_tiles[c][:, n].rearrange("p h w -> p (h w)")
                # u = (x >= -b0) in {0.0, 1.0}; then 2u-1 -> {-1, +1} fp8
                nc.vector.tensor_scalar(
                    out=u, in0=xin, scalar1=negb0_c[c], scalar2=None,
                    op0=mybir.AluOpType.is_ge,
                )
                dst = act_q[c // 2][:, c % 2, n, : HP * WP].rearrange(
                    "p (h w) -> p h w", w=WP
                )[:, 1 : 1 + H, 1 : 1 + W]
                nc.vector.tensor_scalar(
                    out=dst,
                    in0=u.rearrange("p (h w) -> p h w", w=W),
                    scalar1=2.0, scalar2=-1.0,
                    op0=mybir.AluOpType.mult, op1=mybir.AluOpType.add,
                )

            # ---- weight prep ---------------------------------------------
            # lhsT : [128(cin), tap, pair, half, cout] fp8 DoubleRow weights
            lhsT = persist.tile(
                [128, TAPS, NPAIR, 2, C], FP8, tag="lhsT", name="lhsT"
            )
            alphabar = {}  # per cout chunk [128,1]: alpha*gain*sf
            wsigns = {}
            mvs = {}

            def weight_prep_a(m):
                """stats + binarize -- the critical path to the transposes"""
                wt = w_tiles[m]
                st = stats.tile([128, TAPS, 6], FP32, tag="bnst", name="bnst")
                wt3 = wt.rearrange("p (a b) -> p a b", b=512)
                for sg in range(TAPS):
                    nc.vector.bn_stats(out=st[:, sg, :], in_=wt3[:, sg, :])
                mv = stats.tile([128, 2], FP32, tag="bnagg", name="bnagg")
                nc.vector.bn_aggr(out=mv, in_=st)

                negmean = stats.tile([128, 1], FP32, tag="negmean", name="negmean")
                nc.vector.tensor_scalar_mul(out=negmean, in0=mv[:, 0:1], scalar1=-1.0)

                # sign(w - mean) -> bf16, split per cin block for finer
                # ACT interleaving
                ws = scratch.tile([128, KFAN], BF16, tag="wsign", name="wsign")
                for b in range(NCH):
                    nc.scalar.activation(
                        out=ws[:, b * 1152 : (b + 1) * 1152],
                        in_=wt[:, b * 1152 : (b + 1) * 1152],
                        func=mybir.ActivationFunctionType.Sign,
                        bias=negmean,
                    )
                wsigns[m] = ws
                mvs[m] = (mv, negmean)

            def weight_prep_b(m, alt_cast=True):
                """transpose to [cin, (pair, half), cout] per tap; 4 cin-block
                transposes share one PSUM bank; casts alternate DVE/ACT
                (all-DVE for m0 where ACT is loaded with signs)"""
                ws3 = wsigns[m].rearrange("p (i t) -> p i t", t=TAPS)
                for t in range(TAPS):
                    ps = psum_tr.tile([128, NCH * 128], BF16, tag="ptr", name="ptr")
                    for b in range(NCH):
                        nc.tensor.transpose(
                            ps[:, b * 128 : (b + 1) * 128],
                            ws3[:, b * 128 : (b + 1) * 128, t],
                            ident,
                        )
                    dst = lhsT[:, t, :, :, m * 128 : (m + 1) * 128]
                    if alt_cast and t % 2 == 1:
                        nc.scalar.activation(
                            out=dst,
                            in_=ps.rearrange("p (q h o) -> p q h o", q=2, h=2),
                            func=mybir.ActivationFunctionType.Identity,
                        )
                    else:
                        nc.vector.tensor_copy(out=dst, in_=ps)

            def weight_prep_c(m):
                """1/(std+eps), sum|w-mean| -> alphabar; off critical path"""
                wt = w_tiles[m]
                mv, negmean = mvs[m]
                stdeps = stats.tile([128, 1], FP32, tag="stdeps", name="stdeps")
                nc.scalar.activation(
                    out=stdeps, in_=mv[:, 1:2], func=mybir.ActivationFunctionType.Sqrt
                )
                nc.vector.tensor_scalar_add(out=stdeps, in0=stdeps, scalar1=EPS)
                inv = stats.tile([128, 1], FP32, tag="inv", name="inv")
                nc.vector.reciprocal(out=inv, in_=stdeps)

                sumabs = stats.tile([128, NCH], FP32, tag="sumabs", name="sumabs")
                for b in range(NCH):
                    nc.scalar.activation(
                        out=wt[:, b * 1152 : (b + 1) * 1152],
                        in_=wt[:, b * 1152 : (b + 1) * 1152],
                        func=mybir.ActivationFunctionType.Abs,
                        bias=negmean,
                        accum_out=sumabs[:, b : b + 1],
                    )
                sumabs1 = stats.tile([128, 1], FP32, tag="sumabs1", name="sumabs1")
                nc.vector.tensor_reduce(
                    out=sumabs1, in_=sumabs, axis=mybir.AxisListType.X,
                    op=mybir.AluOpType.add,
                )

                ab = persist.tile(
                    [128, 1], FP32, tag=f"alphabar{m}", name=f"alphabar{m}"
                )
                nc.vector.tensor_tensor(
                    out=ab, in0=sumabs1, in1=inv, op=mybir.AluOpType.mult
                )
                nc.vector.tensor_tensor(
                    out=ab, in0=ab, in1=gain_c[m], op=mybir.AluOpType.mult
                )
                nc.vector.tensor_scalar_mul(
                    out=ab, in0=ab, scalar1=ALPHA * WS_SCALE / KFAN
                )
                alphabar[m] = ab

            def conv_block(m, mid_cb=None):
                for n in range(N_PER):
                    if n == 3 and mid_cb is not None:
                        mid_cb()
                    # both spatial tiles of image n accumulate together,
                    # q-outer so tile 0 can start before chunks 2/3 arrive
                    accs = [
                        psum_mm.tile([128, NFREE], FP32, tag="acc", name="acc")
                        for _ in range(NSPAT)
                    ]
                    for q in range(NPAIR):
                        for t in range(TAPS):
                            dy, dx = t // 3, t % 3
                            for h2 in range(NSPAT):
                                y0 = h2 * ROWS_PER_TILE
                                base = (y0 + dy) * WP + dx
                                rhs = act_q[q][:, :, n, base : base + NFREE]
                                i = q * TAPS + t
                                nc.tensor.matmul(
                                    accs[h2],
                                    lhsT[:, t, q, :, m * 128 : (m + 1) * 128],
                                    rhs,
                                    start=(i == 0),
                                    stop=(i == NPAIR * TAPS - 1),
                                    perf_mode=mybir.MatmulPerfMode.DoubleRow,
                                )
                    # epilogue into a per-(m,n) staging tile, one DMA out
                    zst = stage.tile(
                        [128, NSPAT, ROWS_PER_TILE, W], FP32, tag="zst",
                        name="zst",
                    )
                    for h2 in range(NSPAT):
                        y0 = h2 * ROWS_PER_TILE
                        accv = accs[h2].rearrange("p (h w) -> p h w", w=WP)[
                            :, :, 0:W
                        ]
                        res = xs_tiles[m][:, n, y0 : y0 + ROWS_PER_TILE, :]
                        # z = acc*alphabar + residual (prelu input minus b1)
                        z = epi.tile(
                            [128, ROWS_PER_TILE, W], FP32, tag="z", name="z"
                        )
                        nc.vector.scalar_tensor_tensor(
                            out=z, in0=accv, scalar=alphabar[m], in1=res,
                            op0=mybir.AluOpType.mult, op1=mybir.AluOpType.add,
                        )
                        # r = relu(z + b1) on ACT
                        r = epi.tile(
                            [128, ROWS_PER_TILE, W], FP32, tag="r", name="r"
                        )
                        nc.scalar.activation(
                            out=r, in_=z,
                            func=mybir.ActivationFunctionType.Relu,
                            bias=b1_c[m],
                        )
                        # zz = a*z + (a*b1 + b2) ; out = (1-a)*r + zz
                        zz = epi.tile(
                            [128, ROWS_PER_TILE, W], FP32, tag="zz", name="zz"
                        )
                        nc.scalar.activation(
                            out=zz, in_=z,
                            func=mybir.ActivationFunctionType.Identity,
                            scale=pa_c[m], bias=ab1b2[m],
                        )
                        nc.vector.scalar_tensor_tensor(
                            out=zst[:, h2], in0=r, scalar=one_minus_a[m],
                            in1=zz,
                            op0=mybir.AluOpType.mult, op1=mybir.AluOpType.add,
                        )
                    nc.gpsimd.dma_start(
                        out=out_ap[n, m * 128 : (m + 1) * 128],
                        in_=zst.rearrange("p s r w -> p (s r) w"),
                    )

            # ---- emission order ------------------------------------------
            # ACT queue: wsign(m0) first (ready at ~14us when w0 lands),
            # then the image-0/1 chunk-2/3 signs, then images 2..3.
            # DVE handles image-0/1 chunks 0/1 in parallel with wsign.
            weight_prep_a(0)
            xsign_dve(0, 0)
            xsign_dve(0, 1)
            xsign_act(0, 2)
            xsign_act(0, 3)
            weight_prep_b(0, alt_cast=False)
            xsign_act(1, 2)
            xsign_act(1, 3)
            xsign_act(1, 1)
            xsign_dve(1, 0)
            for n in range(2, N_PER):
                for c in range(NCH):
                    xsign_act(n, c)
            weight_prep_c(0)

            for m in range(NCH):
                if m + 1 < NCH:
                    mid = lambda mm=m + 1: (weight_prep_a(mm), weight_prep_c(mm))
                else:
                    mid = None
                conv_block(m, mid_cb=mid)
                if m + 1 < NCH:
                    weight_prep_b(m + 1)

    nc.finalize()
    return nc


_NC_CACHE = None


def _get_program():
    global _NC_CACHE
    if _NC_CACHE is None:
        _NC_CACHE = build_program()
    return _NC_CACHE


def kernel(**inputs):
    from concourse.bass_utils import run_bass_kernel_spmd

    x = np.ascontiguousarray(np.asarray(inputs["x"], dtype=np.float32))
    shared = {
        name: np.ascontiguousarray(np.asarray(inputs[name], dtype=np.float32))
        for name in (
            "conv_weight", "gain", "move0_bias", "move1_bias", "prelu_a",
            "move2_bias",
        )
    }
    nc = _get_program()
    in_maps = [
        {"x": x[i * N_PER : (i + 1) * N_PER], **shared} for i in range(N_CORES)
    ]
    res = run_bass_kernel_spmd(nc, in_maps, core_ids=list(range(N_CORES)))
    return np.concatenate([r["out"] for r in res.results], axis=0)
